# revision 23
# baseline (speedup 1.0000x reference)
"""GBST Trainium2 kernel (nn_GBST_42434276884940).

Self-contained: takes FULL inputs, shards batch over 8 NeuronCores
(2 rows/core), runs a Bass/Tile kernel per core, gathers full output.

Device algorithm per core (6144 positions = 48 chunks of 128):
- One dma_gather per 12-chunk group from a host-built augmented table
  T4pad[v + 256*phi] = [table[v]+pe[phi] (256), (table[v]+pe[phi])@w (1), pad]
  -> X [128, 48, 320] (position g at partition g%128, chunk g//128).
- Scores: pooled per-position scores for block sizes 1..4 via small
  matmuls with constant block-pooling matrices; softmax + tiny
  self-attention calibration on [128,12,4] tiles -> combine weights c4.
- The whole pool->repeat->weighted-combine->downsample(2) stack is one
  banded linear operator G (band ~[2t-3, 2t+4]). Its 128x128 chunk tiles
  (lhsT "Gsb") are built on the PE from C[l, j] = c4 * (j==p//2) masks,
  then applied: out2[t, h] = sum_k Gsb[k, t] * X[k, h] (fp32r matmuls).
"""

import os
import sys

import numpy as np

if "/opt/trn_rl_repo" not in sys.path:
    sys.path.insert(0, "/opt/trn_rl_repo")

import concourse.bass as bass
import concourse.tile as tile
from concourse import bacc, library_config, mybir
from concourse.bass_utils import run_bass_kernel_spmd

F32 = mybir.dt.float32
F32R = mybir.dt.float32r
I16 = mybir.dt.int16
I32 = mybir.dt.int32

MAX_BLOCK = 4
EMBED = 256
VOCAB = 256
BATCH = 16
SEQ = 3072
NCORES = 8
BLOC = BATCH // NCORES           # 2
NPOS = BLOC * SEQ                # 6144
NCHUNK = NPOS // 128             # 48
NGROUP = 4
GSZ = NCHUNK // NGROUP           # 12
ELEM = 320

SLOTS = [0, 3, 6, 9, 2, 5, 8, 11, 1, 4, 7, 10]   # slot s -> tau_l
SLOT_OF = {t: s for s, t in enumerate(SLOTS)}
CLASS_TAUL = [[0, 3, 6, 9], [2, 5, 8, 11], [1, 4, 7, 10]]


# ---------------------------------------------------------------- host consts

def _sinusoidal_pe(max_len, d):
    pos = np.arange(max_len, dtype=np.float32)[:, None]
    div = np.exp(np.arange(0, d, 2, dtype=np.float32) * (-np.log(10000.0) / d))
    pe = np.zeros((max_len, d), dtype=np.float32)
    pe[:, 0::2] = np.sin(pos * div)
    pe[:, 1::2] = np.cos(pos * div)
    return pe


def build_t4pad(embed_table, w_score):
    table = np.asarray(embed_table, dtype=np.float32)
    w = np.asarray(w_score, dtype=np.float32).reshape(EMBED)
    pe = _sinusoidal_pe(MAX_BLOCK, EMBED)
    t4 = np.zeros((MAX_BLOCK * VOCAB, ELEM), dtype=np.float32)
    for phi in range(MAX_BLOCK):
        rows = table + pe[phi][None, :]
        t4[phi * VOCAB:(phi + 1) * VOCAB, :EMBED] = rows
        t4[phi * VOCAB:(phi + 1) * VOCAB, EMBED] = rows @ w
    return t4


def phi_of_taul(tau_l):
    return (2 * tau_l) % 3


def build_smats():
    k = np.arange(128)
    mats = np.zeros((12, 128, 128), dtype=np.float32)
    mats[0] = 0.5 * np.eye(128, dtype=np.float32)
    mats[1] = 0.25 * (k[:, None] // 2 == k[None, :] // 2)
    mats[2] = 0.125 * (k[:, None] // 4 == k[None, :] // 4)
    for phi in range(3):
        mats[3 + phi] = (1 / 6) * ((k[:, None] + phi) // 3 == (k[None, :] + phi) // 3)
        mats[6 + phi] = (1 / 6) * ((128 + k[:, None] + phi) // 3 == (k[None, :] + phi) // 3)
        mats[9 + phi] = (1 / 6) * ((k[:, None] - 128 + phi) // 3 == (k[None, :] + phi) // 3)
    return mats


def build_m2mask():
    p = np.arange(128)
    j = np.arange(64)
    return (j[None, :] == p[:, None] // 2).astype(np.float32)


def build_idx_streams(input_ids):
    """Per-core int32 [128, 48]: idx[p, c] = augmented-table row for
    position g = 128*c + p (indirect-DMA gather: 320 elems per index)."""
    ids = np.asarray(input_ids).astype(np.int64)
    out = []
    g = np.arange(NPOS)
    row, l = g // SEQ, g % SEQ
    for core in range(NCORES):
        core_ids = ids[core * BLOC:(core + 1) * BLOC]
        vid = (core_ids[row, l] + 256 * (l % 4)).astype(np.int32)
        out.append(vid.reshape(NCHUNK, 128).T.copy())
    return out


# ---------------------------------------------------------------- device prog

def _r(ap):
    return ap


def emit_program(nc):
    t4pad_d = nc.dram_tensor("t4pad", [MAX_BLOCK * VOCAB, ELEM], F32,
                             kind="ExternalInput")
    idxs_d = nc.dram_tensor("idxs", [128, NCHUNK], I32,
                            kind="ExternalInput")
    smats_d = nc.dram_tensor("smats", [12, 128, 128], F32, kind="ExternalInput")
    m2_d = nc.dram_tensor("m2m", [128, 64], F32, kind="ExternalInput")
    out_d = nc.dram_tensor("out", [BLOC * SEQ // 2, EMBED], F32,
                           kind="ExternalOutput")

    with tile.TileContext(nc) as tc:
        with (
            tc.tile_pool(name="consts", bufs=1) as consts,
            tc.tile_pool(name="big", bufs=1) as big,
            tc.tile_pool(name="sm", bufs=2) as sm,
            tc.tile_pool(name="outsb", bufs=4) as outsb_pool,
            tc.tile_pool(name="scT_ps", bufs=1, space="PSUM") as scT_ps,
            tc.tile_pool(name="gmid_ps", bufs=1, space="PSUM") as gmid_ps,
            tc.tile_pool(name="glow_ps", bufs=1, space="PSUM") as glow_ps,
            tc.tile_pool(name="ghigh_ps", bufs=1, space="PSUM") as ghigh_ps,
            tc.tile_pool(name="out2_ps", bufs=2, space="PSUM") as out2_ps,
        ):
            # ---- constants to SBUF ----
            smats_sb = consts.tile([128, 12, 128], F32, tag="smats")
            nc.sync.dma_start(
                smats_sb[:],
                bass.AP(tensor=smats_d, offset=0,
                        ap=[[128, 128], [128 * 128, 12], [1, 128]]))
            m2_sb = consts.tile([128, 64], F32, tag="m2")
            nc.sync.dma_start(m2_sb[:], m2_d.ap()[:, :])
            idxs_sb = consts.tile([128, NCHUNK], I32, tag="ix")
            nc.sync.dma_start(idxs_sb[:], idxs_d.ap()[:, :])

            # ---- persistent big tensors ----
            X = big.tile([128, NCHUNK, ELEM], F32, tag="X")
            C = big.tile([128, NCHUNK, 4, 64], F32, tag="C")
            Gsb = big.tile([128, NCHUNK, 128], F32, tag="Gsb")
            d2 = big.tile([128, NCHUNK], F32, tag="d2")   # slot-ordered
            c4 = big.tile([128, NCHUNK, 4], F32, tag="c4")  # slot-ordered

            def mmat(out_ap, mi, rhs_ap, start, stop, f32r=False):
                # exact fp32 everywhere: fp32r measured at ~2e-4 rel err
                # (TF32-class), too coarse for this problem.
                lhsT = smats_sb[:, mi, :].bitcast(F32)
                rhs = rhs_ap.bitcast(F32)
                nc.tensor.matmul(out=out_ap, lhsT=lhsT, rhs=rhs,
                                 start=start, stop=stop,
                                 skip_group_check=True)

            def emit_gather(g):
                g0 = g * GSZ
                # multi-index indirect DMA is broken on HW: one call per
                # chunk ([128,1] indices -> 128 rows x 1280 B)
                for c in range(GSZ):
                    nc.gpsimd.indirect_dma_start(
                        out=X[:, g0 + c, :], out_offset=None,
                        in_=t4pad_d.ap()[:, :],
                        in_offset=bass.IndirectOffsetOnAxis(
                            ap=idxs_sb[:, g0 + c:g0 + c + 1], axis=0))
                # d2 = 2*d (score path feeds half-scaled matrices),
                # written in slot order: class c block <- tau_l stride-3 run
                for c in range(3):
                    t0 = CLASS_TAUL[c][0]
                    nc.scalar.mul(
                        d2[:, g0 + 4 * c:g0 + 4 * c + 4].unsqueeze(2),
                        X[:, g0 + t0:g0 + t0 + 10:3, EMBED:EMBED + 1], 2.0)

            def emit_scores(g):
                g0 = g * GSZ
                # m-major psum layout [128, 4 m, 12 slot]; every MM writes a
                # contiguous slot run (slot space makes up/dn sources
                # contiguous class blocks too).
                scT = scT_ps.tile([128, 4, GSZ], F32, tag="scT")
                mmat(scT[:, 0, :], 0, d2[:, g0:g0 + GSZ], True, False, False)
                mmat(scT[:, 1, :], 1, d2[:, g0:g0 + GSZ], False, False, False)
                mmat(scT[:, 3, :], 2, d2[:, g0:g0 + GSZ], False, False, False)
                # m=3 diag: slot block 4c:4c+4 <- same slots
                for c in range(3):
                    phi = phi_of_taul(CLASS_TAUL[c][0])
                    mmat(scT[:, 2, 4 * c:4 * c + 4], 3 + phi,
                         d2[:, g0 + 4 * c:g0 + 4 * c + 4], False, False, False)
                # up: (class c out slots, source slots); dn likewise
                up_sc = [(0, 0, 4, 8), (1, 4, 3, 1), (2, 8, 4, 4)]
                dn_sc = [(0, 1, 3, 4), (1, 4, 4, 8), (2, 8, 4, 0)]
                for plan, base in ((up_sc, 6), (dn_sc, 9)):
                    for c, o0, on, s0 in plan:
                        phi = phi_of_taul(CLASS_TAUL[c][0])
                        mmat(scT[:, 2, o0:o0 + on], base + phi,
                             d2[:, g0 + s0:g0 + s0 + on], False, False, False)
                if g % 2 == 0:   # up-fix: slot 7 (tau_l 11) <- next grp slot 0
                    mmat(scT[:, 2, 7:8], 6 + phi_of_taul(11),
                         d2[:, (g + 1) * GSZ:(g + 1) * GSZ + 1], False, False,
                         False)
                else:            # dn-fix: slot 0 <- prev group slot 7
                    mmat(scT[:, 2, 0:1], 9 + phi_of_taul(0),
                         d2[:, g0 - GSZ + 7:g0 - GSZ + 8], False, True, False)

                # softmax + calibration (scS transposed to [128, slot, m])
                scS = sm.tile([128, GSZ, 4], F32, tag="scS")
                base_ap = scT[:]
                scT_t = bass.AP(tensor=base_ap.tensor, offset=base_ap.offset,
                                ap=[list(base_ap.ap[0]), list(base_ap.ap[2]),
                                    list(base_ap.ap[1])])
                nc.vector.tensor_copy(out=scS[:], in_=scT_t)
                ex = sm.tile([128, GSZ, 4], F32, tag="ex")
                nc.scalar.activation(out=ex[:], in_=scS[:],
                                     func=mybir.ActivationFunctionType.Exp)
                Z = sm.tile([128, GSZ], F32, tag="Z")
                nc.vector.tensor_reduce(out=Z[:], in_=ex[:],
                                        axis=mybir.AxisListType.X,
                                        op=mybir.AluOpType.add)
                rz = sm.tile([128, GSZ], F32, tag="rz")
                nc.vector.reciprocal(out=rz[:], in_=Z[:])
                r = sm.tile([128, GSZ, 4], F32, tag="r")
                nc.vector.tensor_tensor(
                    out=r[:], in0=ex[:],
                    in1=rz[:].unsqueeze(2).to_broadcast([128, GSZ, 4]),
                    op=mybir.AluOpType.mult)
                P = sm.tile([128, GSZ, 4, 4], F32, tag="P")
                nc.vector.tensor_tensor(
                    out=P[:],
                    in0=r[:].unsqueeze(3).to_broadcast([128, GSZ, 4, 4]),
                    in1=r[:].unsqueeze(2).to_broadcast([128, GSZ, 4, 4]),
                    op=mybir.AluOpType.mult)
                E = sm.tile([128, GSZ, 4, 4], F32, tag="E")
                nc.scalar.activation(out=E[:], in_=P[:],
                                     func=mybir.ActivationFunctionType.Exp)
                D = sm.tile([128, GSZ, 4], F32, tag="D")
                nc.vector.tensor_reduce(out=D[:], in_=E[:],
                                        axis=mybir.AxisListType.X,
                                        op=mybir.AluOpType.add)
                EN = sm.tile([128, GSZ, 4, 4], F32, tag="EN")
                nc.vector.tensor_tensor(
                    out=EN[:], in0=E[:],
                    in1=r[:].unsqueeze(2).to_broadcast([128, GSZ, 4, 4]),
                    op=mybir.AluOpType.mult)
                Nn = sm.tile([128, GSZ, 4], F32, tag="Nn")
                nc.vector.tensor_reduce(out=Nn[:], in_=EN[:],
                                        axis=mybir.AxisListType.X,
                                        op=mybir.AluOpType.add)
                rD = sm.tile([128, GSZ, 4], F32, tag="rD")
                nc.vector.reciprocal(out=rD[:], in_=D[:])
                nc.vector.tensor_tensor(out=c4[:, g0:g0 + GSZ, :], in0=Nn[:],
                                        in1=rD[:], op=mybir.AluOpType.mult)

                # C build (c4 already slot-ordered -> one TT)
                nc.vector.tensor_tensor(
                    out=C[:, g0:g0 + GSZ, :, :],
                    in0=c4[:, g0:g0 + GSZ, :].to_broadcast([128, GSZ, 4, 64]),
                    in1=m2_sb[:].unsqueeze(1).unsqueeze(1).to_broadcast(
                        [128, GSZ, 4, 64]),
                    op=mybir.AluOpType.mult)

            def emit_builds(g):
                g0 = g * GSZ
                gm = gmid_ps.tile([128, GSZ, 64], F32, tag="gm")
                # m2 / m4 over slots 0-7 then 8-11 (bank split)
                mmat(gm[:, 0:8, :], 1, C[:, g0:g0 + 8, 1, :], True, False)
                mmat(gm[:, 8:12, :], 1, C[:, g0 + 8:g0 + 12, 1, :], True, False)
                mmat(gm[:, 0:8, :], 2, C[:, g0:g0 + 8, 3, :], False, False)
                mmat(gm[:, 8:12, :], 2, C[:, g0 + 8:g0 + 12, 3, :], False, False)
                for c in range(3):
                    phi = phi_of_taul(CLASS_TAUL[c][0])
                    mmat(gm[:, 4 * c:4 * c + 4, :], 3 + phi,
                         C[:, g0 + 4 * c:g0 + 4 * c + 4, 2, :], False, True)
                gl = glow_ps.tile([128, GSZ, 32], F32, tag="gl")
                dn_plan = [(0, 1, 3, 4), (1, 4, 4, 8), (2, 8, 4, 0)]
                for i, (c, o0, on, s0) in enumerate(dn_plan):
                    phi = phi_of_taul(CLASS_TAUL[c][0])
                    mmat(gl[:, o0:o0 + on, :], 9 + phi,
                         C[:, g0 + s0:g0 + s0 + on, 2, 32:64], i == 0, True)
                if g % 2 == 1:
                    mmat(gl[:, 0:1, :], 9 + phi_of_taul(0),
                         C[:, g0 - 12 + 7:g0 - 12 + 8, 2, 32:64], False, True)
                gh = ghigh_ps.tile([128, GSZ, 32], F32, tag="gh")
                up_plan = [(0, 0, 4, 8), (1, 4, 3, 1), (2, 8, 4, 4)]
                for i, (c, o0, on, s0) in enumerate(up_plan):
                    phi = phi_of_taul(CLASS_TAUL[c][0])
                    mmat(gh[:, o0:o0 + on, :], 6 + phi,
                         C[:, g0 + s0:g0 + s0 + on, 2, 0:32], i == 0, True)
                if g % 2 == 0:
                    mmat(gh[:, 7:8, :], 6 + phi_of_taul(11),
                         C[:, g0 + 12:g0 + 13, 2, 0:32], False, True)
                    nc.vector.memset(gl[:, 0, :], 0.0)
                else:
                    nc.vector.memset(gh[:, 7, :], 0.0)
                # assemble Gsb
                nc.vector.scalar_tensor_tensor(
                    out=Gsb[:, g0:g0 + GSZ, 32:96],
                    in0=C[:, g0:g0 + GSZ, 0, :], scalar=0.5, in1=gm[:],
                    op0=mybir.AluOpType.mult, op1=mybir.AluOpType.add)
                nc.scalar.copy(Gsb[:, g0:g0 + GSZ, 0:32], gl[:])
                nc.scalar.copy(Gsb[:, g0:g0 + GSZ, 96:128], gh[:])

            def gsb_idx(row, tt):
                g = 2 * row + tt // GSZ
                return g * GSZ + SLOT_OF[tt % GSZ]

            def emit_big(row, ot_list):
                for ot in ot_list:
                    out2 = out2_ps.tile([128, EMBED], F32, tag="out2")
                    tt_e = 2 * ot
                    if tt_e < 24:
                        nc.tensor.matmul(
                            out=out2[:, :],
                            lhsT=Gsb[:, gsb_idx(row, tt_e), :].bitcast(F32),
                            rhs=X[:, 24 * row + tt_e, 0:EMBED].bitcast(F32),
                            start=True, stop=False, skip_group_check=True)
                    if tt_e - 1 >= 0:
                        nc.tensor.matmul(
                            out=out2[0:64, :],
                            lhsT=Gsb[:, gsb_idx(row, tt_e - 1), 64:128].bitcast(F32),
                            rhs=X[:, 24 * row + tt_e - 1, 0:EMBED].bitcast(F32),
                            start=(tt_e >= 24), stop=True,
                            skip_group_check=True)
                    if tt_e + 1 < 24:
                        # fp32r matmul needs out base_partition 0; this one
                        # targets partitions 64:128 -> plain fp32 (4 cyc/row)
                        nc.tensor.matmul(
                            out=out2[64:128, :],
                            lhsT=Gsb[:, gsb_idx(row, tt_e + 1), 0:64].bitcast(F32),
                            rhs=X[:, 24 * row + tt_e + 1, 0:EMBED].bitcast(F32),
                            start=False, stop=True, skip_group_check=True)
                    osb = outsb_pool.tile([128, EMBED], F32, tag="osb")
                    p0, p1 = (32, 128) if ot == 0 else (0, 32) if ot == 12 \
                        else (0, 128)
                    # copies start at partition 0 (engine partition-base rule);
                    # ot==0 copies garbage rows 0:32 too, DMA skips them.
                    c0, c1 = (0, 32) if ot == 12 else (0, 128)
                    if ot % 2 == 0:
                        nc.vector.tensor_copy(out=osb[c0:c1, :],
                                              in_=out2[c0:c1, :])
                    else:
                        nc.scalar.copy(osb[c0:c1, :], out2[c0:c1, :])
                    base = row * (SEQ // 2) + 128 * ot - 32
                    nc.sync.dma_start(out_d.ap()[base + p0:base + p1, :],
                                      osb[p0:p1, :])

            # ---- staged pipeline ----
            emit_gather(0)
            emit_gather(1)
            emit_scores(0)
            emit_gather(2)
            emit_scores(1)
            emit_builds(0)
            emit_big(0, list(range(0, 6)))
            emit_gather(3)
            emit_scores(2)
            emit_builds(1)
            emit_big(0, list(range(6, 13)))
            emit_scores(3)
            emit_builds(2)
            emit_big(1, list(range(0, 6)))
            emit_builds(3)
            emit_big(1, list(range(6, 13)))

    return nc


_CACHE = {}


def _get_nc():
    if "nc" not in _CACHE:
        nc = bacc.Bacc("TRN2", target_bir_lowering=False, debug=False)
        emit_program(nc)
        nc.compile()
        _CACHE["nc"] = nc
    return _CACHE["nc"]


def prepare_in_maps(input_ids, embed_table, w_score, b_score=None):
    # b_score only shifts all 4 scores equally -> softmax-invariant; unused.
    t4pad = build_t4pad(embed_table, w_score)
    smats = build_smats()
    m2 = build_m2mask()
    idx_streams = build_idx_streams(input_ids)
    return [{"t4pad": t4pad, "idxs": idx_streams[core],
             "smats": smats, "m2m": m2} for core in range(NCORES)]


def assemble_out(results):
    outs = [results[c]["out"].reshape(BLOC, SEQ // 2, EMBED)
            for c in range(NCORES)]
    return np.concatenate(outs, axis=0)


def kernel(input_ids, embed_table, w_score, b_score):
    in_maps = prepare_in_maps(input_ids, embed_table, w_score, b_score)
    res = run_bass_kernel_spmd(_get_nc(), in_maps,
                               core_ids=list(range(NCORES)))
    return assemble_out(res.results)


# revision 24
# speedup vs baseline: 65.6824x; 65.6824x over previous
"""GBST Trainium2 kernel (nn_GBST_42434276884940).

Self-contained: takes FULL inputs, shards batch over 8 NeuronCores
(2 rows/core), runs a Bass/Tile kernel per core, gathers full output.

Device algorithm per core (6144 positions = 48 chunks of 128):
- One dma_gather per 12-chunk group from a host-built augmented table
  T4pad[v + 256*phi] = [table[v]+pe[phi] (256), (table[v]+pe[phi])@w (1), pad]
  -> X [128, 48, 320] (position g at partition g%128, chunk g//128).
- Scores: pooled per-position scores for block sizes 1..4 via small
  matmuls with constant block-pooling matrices; softmax + tiny
  self-attention calibration on [128,12,4] tiles -> combine weights c4.
- The whole pool->repeat->weighted-combine->downsample(2) stack is one
  banded linear operator G (band ~[2t-3, 2t+4]). Its 128x128 chunk tiles
  (lhsT "Gsb") are built on the PE from C[l, j] = c4 * (j==p//2) masks,
  then applied: out2[t, h] = sum_k Gsb[k, t] * X[k, h] (fp32r matmuls).
"""

import os
import sys

import numpy as np

if "/opt/trn_rl_repo" not in sys.path:
    sys.path.insert(0, "/opt/trn_rl_repo")

import concourse.bass as bass
import concourse.tile as tile
from concourse import bacc, library_config, mybir
from concourse.bass_utils import run_bass_kernel_spmd

F32 = mybir.dt.float32
F32R = mybir.dt.float32r
I16 = mybir.dt.int16
I32 = mybir.dt.int32

MAX_BLOCK = 4
EMBED = 256
VOCAB = 256
BATCH = 16
SEQ = 3072
NCORES = 8
BLOC = BATCH // NCORES           # 2
NPOS = BLOC * SEQ                # 6144
NCHUNK = NPOS // 128             # 48
NGROUP = 4
GSZ = NCHUNK // NGROUP           # 12
ELEM = 320

SLOTS = [0, 3, 6, 9, 2, 5, 8, 11, 1, 4, 7, 10]   # slot s -> tau_l
SLOT_OF = {t: s for s, t in enumerate(SLOTS)}
CLASS_TAUL = [[0, 3, 6, 9], [2, 5, 8, 11], [1, 4, 7, 10]]


# ---------------------------------------------------------------- host consts

def _sinusoidal_pe(max_len, d):
    pos = np.arange(max_len, dtype=np.float32)[:, None]
    div = np.exp(np.arange(0, d, 2, dtype=np.float32) * (-np.log(10000.0) / d))
    pe = np.zeros((max_len, d), dtype=np.float32)
    pe[:, 0::2] = np.sin(pos * div)
    pe[:, 1::2] = np.cos(pos * div)
    return pe


def build_t4pad(embed_table, w_score):
    table = np.asarray(embed_table, dtype=np.float32)
    w = np.asarray(w_score, dtype=np.float32).reshape(EMBED)
    pe = _sinusoidal_pe(MAX_BLOCK, EMBED)
    t4 = np.zeros((MAX_BLOCK * VOCAB, ELEM), dtype=np.float32)
    for phi in range(MAX_BLOCK):
        rows = table + pe[phi][None, :]
        t4[phi * VOCAB:(phi + 1) * VOCAB, :EMBED] = rows
        t4[phi * VOCAB:(phi + 1) * VOCAB, EMBED] = rows @ w
    return t4


def phi_of_taul(tau_l):
    return (2 * tau_l) % 3


def build_smats():
    k = np.arange(128)
    mats = np.zeros((12, 128, 128), dtype=np.float32)
    mats[0] = 0.5 * np.eye(128, dtype=np.float32)
    mats[1] = 0.25 * (k[:, None] // 2 == k[None, :] // 2)
    mats[2] = 0.125 * (k[:, None] // 4 == k[None, :] // 4)
    for phi in range(3):
        mats[3 + phi] = (1 / 6) * ((k[:, None] + phi) // 3 == (k[None, :] + phi) // 3)
        mats[6 + phi] = (1 / 6) * ((128 + k[:, None] + phi) // 3 == (k[None, :] + phi) // 3)
        mats[9 + phi] = (1 / 6) * ((k[:, None] - 128 + phi) // 3 == (k[None, :] + phi) // 3)
    return mats


def build_m2mask():
    p = np.arange(128)
    j = np.arange(64)
    return (j[None, :] == p[:, None] // 2).astype(np.float32)


def build_idx_streams(input_ids):
    """Per-core int32 [128, 48]: idx[p, c] = augmented-table row for
    position g = 128*c + p (indirect-DMA gather: 320 elems per index)."""
    ids = np.asarray(input_ids).astype(np.int64)
    out = []
    g = np.arange(NPOS)
    row, l = g // SEQ, g % SEQ
    for core in range(NCORES):
        core_ids = ids[core * BLOC:(core + 1) * BLOC]
        vid = (core_ids[row, l] + 256 * (l % 4)).astype(np.int32)
        out.append(vid.reshape(NCHUNK, 128).T.copy())
    return out


# ---------------------------------------------------------------- device prog

def _r(ap):
    return ap


def emit_program(nc, nrep=1):
    t4pad_d = nc.dram_tensor("t4pad", [MAX_BLOCK * VOCAB, ELEM], F32,
                             kind="ExternalInput")
    idxs_d = nc.dram_tensor("idxs", [128, NCHUNK], I32,
                            kind="ExternalInput")
    smats_d = nc.dram_tensor("smats", [12, 128, 128], F32, kind="ExternalInput")
    m2_d = nc.dram_tensor("m2m", [128, 64], F32, kind="ExternalInput")
    out_d = nc.dram_tensor("out", [BLOC * SEQ // 2, EMBED], F32,
                           kind="ExternalOutput")

    with tile.TileContext(nc) as tc:
        with (
            tc.tile_pool(name="consts", bufs=1) as consts,
            tc.tile_pool(name="big", bufs=1) as big,
            tc.tile_pool(name="sm", bufs=2) as sm,
            tc.tile_pool(name="outsb", bufs=4) as outsb_pool,
            tc.tile_pool(name="scT_ps", bufs=1, space="PSUM") as scT_ps,
            tc.tile_pool(name="gmid_ps", bufs=1, space="PSUM") as gmid_ps,
            tc.tile_pool(name="glow_ps", bufs=1, space="PSUM") as glow_ps,
            tc.tile_pool(name="ghigh_ps", bufs=1, space="PSUM") as ghigh_ps,
            tc.tile_pool(name="out2_ps", bufs=2, space="PSUM") as out2_ps,
        ):
            # ---- constants to SBUF ----
            smats_sb = consts.tile([128, 12, 128], F32, tag="smats")
            nc.sync.dma_start(
                smats_sb[:],
                bass.AP(tensor=smats_d, offset=0,
                        ap=[[128, 128], [128 * 128, 12], [1, 128]]))
            m2_sb = consts.tile([128, 64], F32, tag="m2")
            nc.sync.dma_start(m2_sb[:], m2_d.ap()[:, :])
            idxs_sb = consts.tile([128, NCHUNK], I32, tag="ix")
            nc.sync.dma_start(idxs_sb[:], idxs_d.ap()[:, :])

            # ---- persistent big tensors ----
            X = big.tile([128, NCHUNK, ELEM], F32, tag="X")
            C = big.tile([128, NCHUNK, 4, 64], F32, tag="C")
            Gsb = big.tile([128, NCHUNK, 128], F32, tag="Gsb")
            d2 = big.tile([128, NCHUNK], F32, tag="d2")   # slot-ordered
            c4 = big.tile([128, NCHUNK, 4], F32, tag="c4")  # slot-ordered

            def mmat(out_ap, mi, rhs_ap, start, stop, f32r=False):
                # exact fp32 everywhere: fp32r measured at ~2e-4 rel err
                # (TF32-class), too coarse for this problem.
                lhsT = smats_sb[:, mi, :].bitcast(F32)
                rhs = rhs_ap.bitcast(F32)
                nc.tensor.matmul(out=out_ap, lhsT=lhsT, rhs=rhs,
                                 start=start, stop=stop,
                                 skip_group_check=True)

            def emit_gather(g):
                g0 = g * GSZ
                # multi-index indirect DMA is broken on HW: one call per
                # chunk ([128,1] indices -> 128 rows x 1280 B)
                for c in range(GSZ):
                    nc.gpsimd.indirect_dma_start(
                        out=X[:, g0 + c, :], out_offset=None,
                        in_=t4pad_d.ap()[:, :],
                        in_offset=bass.IndirectOffsetOnAxis(
                            ap=idxs_sb[:, g0 + c:g0 + c + 1], axis=0))
                # d2 = 2*d (score path feeds half-scaled matrices),
                # written in slot order: class c block <- tau_l stride-3 run
                for c in range(3):
                    t0 = CLASS_TAUL[c][0]
                    nc.scalar.mul(
                        d2[:, g0 + 4 * c:g0 + 4 * c + 4].unsqueeze(2),
                        X[:, g0 + t0:g0 + t0 + 10:3, EMBED:EMBED + 1], 2.0)

            def emit_scores(g):
                g0 = g * GSZ
                # m-major psum layout [128, 4 m, 12 slot]; every MM writes a
                # contiguous slot run (slot space makes up/dn sources
                # contiguous class blocks too).
                scT = scT_ps.tile([128, 4, GSZ], F32, tag="scT")
                mmat(scT[:, 0, :], 0, d2[:, g0:g0 + GSZ], True, False, False)
                mmat(scT[:, 1, :], 1, d2[:, g0:g0 + GSZ], False, False, False)
                mmat(scT[:, 3, :], 2, d2[:, g0:g0 + GSZ], False, False, False)
                # m=3 diag: slot block 4c:4c+4 <- same slots
                for c in range(3):
                    phi = phi_of_taul(CLASS_TAUL[c][0])
                    mmat(scT[:, 2, 4 * c:4 * c + 4], 3 + phi,
                         d2[:, g0 + 4 * c:g0 + 4 * c + 4], False, False, False)
                # up: (class c out slots, source slots); dn likewise
                up_sc = [(0, 0, 4, 8), (1, 4, 3, 1), (2, 8, 4, 4)]
                dn_sc = [(0, 1, 3, 4), (1, 4, 4, 8), (2, 8, 4, 0)]
                for plan, base in ((up_sc, 6), (dn_sc, 9)):
                    for c, o0, on, s0 in plan:
                        phi = phi_of_taul(CLASS_TAUL[c][0])
                        mmat(scT[:, 2, o0:o0 + on], base + phi,
                             d2[:, g0 + s0:g0 + s0 + on], False, False, False)
                if g % 2 == 0:   # up-fix: slot 7 (tau_l 11) <- next grp slot 0
                    mmat(scT[:, 2, 7:8], 6 + phi_of_taul(11),
                         d2[:, (g + 1) * GSZ:(g + 1) * GSZ + 1], False, False,
                         False)
                else:            # dn-fix: slot 0 <- prev group slot 7
                    mmat(scT[:, 2, 0:1], 9 + phi_of_taul(0),
                         d2[:, g0 - GSZ + 7:g0 - GSZ + 8], False, True, False)

                # softmax + calibration (scS transposed to [128, slot, m])
                scS = sm.tile([128, GSZ, 4], F32, tag="scS")
                base_ap = scT[:]
                scT_t = bass.AP(tensor=base_ap.tensor, offset=base_ap.offset,
                                ap=[list(base_ap.ap[0]), list(base_ap.ap[2]),
                                    list(base_ap.ap[1])])
                nc.vector.tensor_copy(out=scS[:], in_=scT_t)
                ex = sm.tile([128, GSZ, 4], F32, tag="ex")
                nc.scalar.activation(out=ex[:], in_=scS[:],
                                     func=mybir.ActivationFunctionType.Exp)
                Z = sm.tile([128, GSZ], F32, tag="Z")
                nc.vector.tensor_reduce(out=Z[:], in_=ex[:],
                                        axis=mybir.AxisListType.X,
                                        op=mybir.AluOpType.add)
                rz = sm.tile([128, GSZ], F32, tag="rz")
                nc.vector.reciprocal(out=rz[:], in_=Z[:])
                r = sm.tile([128, GSZ, 4], F32, tag="r")
                nc.vector.tensor_tensor(
                    out=r[:], in0=ex[:],
                    in1=rz[:].unsqueeze(2).to_broadcast([128, GSZ, 4]),
                    op=mybir.AluOpType.mult)
                P = sm.tile([128, GSZ, 4, 4], F32, tag="P")
                nc.vector.tensor_tensor(
                    out=P[:],
                    in0=r[:].unsqueeze(3).to_broadcast([128, GSZ, 4, 4]),
                    in1=r[:].unsqueeze(2).to_broadcast([128, GSZ, 4, 4]),
                    op=mybir.AluOpType.mult)
                E = sm.tile([128, GSZ, 4, 4], F32, tag="E")
                nc.scalar.activation(out=E[:], in_=P[:],
                                     func=mybir.ActivationFunctionType.Exp)
                D = sm.tile([128, GSZ, 4], F32, tag="D")
                nc.vector.tensor_reduce(out=D[:], in_=E[:],
                                        axis=mybir.AxisListType.X,
                                        op=mybir.AluOpType.add)
                EN = sm.tile([128, GSZ, 4, 4], F32, tag="EN")
                nc.vector.tensor_tensor(
                    out=EN[:], in0=E[:],
                    in1=r[:].unsqueeze(2).to_broadcast([128, GSZ, 4, 4]),
                    op=mybir.AluOpType.mult)
                Nn = sm.tile([128, GSZ, 4], F32, tag="Nn")
                nc.vector.tensor_reduce(out=Nn[:], in_=EN[:],
                                        axis=mybir.AxisListType.X,
                                        op=mybir.AluOpType.add)
                rD = sm.tile([128, GSZ, 4], F32, tag="rD")
                nc.vector.reciprocal(out=rD[:], in_=D[:])
                nc.vector.tensor_tensor(out=c4[:, g0:g0 + GSZ, :], in0=Nn[:],
                                        in1=rD[:], op=mybir.AluOpType.mult)

                # C build (c4 already slot-ordered -> one TT)
                nc.vector.tensor_tensor(
                    out=C[:, g0:g0 + GSZ, :, :],
                    in0=c4[:, g0:g0 + GSZ, :].to_broadcast([128, GSZ, 4, 64]),
                    in1=m2_sb[:].unsqueeze(1).unsqueeze(1).to_broadcast(
                        [128, GSZ, 4, 64]),
                    op=mybir.AluOpType.mult)

            def emit_builds(g):
                g0 = g * GSZ
                gm = gmid_ps.tile([128, GSZ, 64], F32, tag="gm")
                # m2 / m4 over slots 0-7 then 8-11 (bank split)
                mmat(gm[:, 0:8, :], 1, C[:, g0:g0 + 8, 1, :], True, False)
                mmat(gm[:, 8:12, :], 1, C[:, g0 + 8:g0 + 12, 1, :], True, False)
                mmat(gm[:, 0:8, :], 2, C[:, g0:g0 + 8, 3, :], False, False)
                mmat(gm[:, 8:12, :], 2, C[:, g0 + 8:g0 + 12, 3, :], False, False)
                for c in range(3):
                    phi = phi_of_taul(CLASS_TAUL[c][0])
                    mmat(gm[:, 4 * c:4 * c + 4, :], 3 + phi,
                         C[:, g0 + 4 * c:g0 + 4 * c + 4, 2, :], False, True)
                gl = glow_ps.tile([128, GSZ, 32], F32, tag="gl")
                dn_plan = [(0, 1, 3, 4), (1, 4, 4, 8), (2, 8, 4, 0)]
                for i, (c, o0, on, s0) in enumerate(dn_plan):
                    phi = phi_of_taul(CLASS_TAUL[c][0])
                    mmat(gl[:, o0:o0 + on, :], 9 + phi,
                         C[:, g0 + s0:g0 + s0 + on, 2, 32:64], i == 0, True)
                if g % 2 == 1:
                    mmat(gl[:, 0:1, :], 9 + phi_of_taul(0),
                         C[:, g0 - 12 + 7:g0 - 12 + 8, 2, 32:64], False, True)
                gh = ghigh_ps.tile([128, GSZ, 32], F32, tag="gh")
                up_plan = [(0, 0, 4, 8), (1, 4, 3, 1), (2, 8, 4, 4)]
                for i, (c, o0, on, s0) in enumerate(up_plan):
                    phi = phi_of_taul(CLASS_TAUL[c][0])
                    mmat(gh[:, o0:o0 + on, :], 6 + phi,
                         C[:, g0 + s0:g0 + s0 + on, 2, 0:32], i == 0, True)
                if g % 2 == 0:
                    mmat(gh[:, 7:8, :], 6 + phi_of_taul(11),
                         C[:, g0 + 12:g0 + 13, 2, 0:32], False, True)
                    nc.vector.memset(gl[:, 0, :], 0.0)
                else:
                    nc.vector.memset(gh[:, 7, :], 0.0)
                # assemble Gsb
                nc.vector.scalar_tensor_tensor(
                    out=Gsb[:, g0:g0 + GSZ, 32:96],
                    in0=C[:, g0:g0 + GSZ, 0, :], scalar=0.5, in1=gm[:],
                    op0=mybir.AluOpType.mult, op1=mybir.AluOpType.add)
                nc.scalar.copy(Gsb[:, g0:g0 + GSZ, 0:32], gl[:])
                nc.scalar.copy(Gsb[:, g0:g0 + GSZ, 96:128], gh[:])

            def gsb_idx(row, tt):
                g = 2 * row + tt // GSZ
                return g * GSZ + SLOT_OF[tt % GSZ]

            def emit_big(row, ot_list):
                for ot in ot_list:
                    out2 = out2_ps.tile([128, EMBED], F32, tag="out2")
                    tt_e = 2 * ot
                    if tt_e < 24:
                        nc.tensor.matmul(
                            out=out2[:, :],
                            lhsT=Gsb[:, gsb_idx(row, tt_e), :].bitcast(F32),
                            rhs=X[:, 24 * row + tt_e, 0:EMBED].bitcast(F32),
                            start=True, stop=False, skip_group_check=True)
                    if tt_e - 1 >= 0:
                        nc.tensor.matmul(
                            out=out2[0:64, :],
                            lhsT=Gsb[:, gsb_idx(row, tt_e - 1), 64:128].bitcast(F32),
                            rhs=X[:, 24 * row + tt_e - 1, 0:EMBED].bitcast(F32),
                            start=(tt_e >= 24), stop=True,
                            skip_group_check=True)
                    if tt_e + 1 < 24:
                        # fp32r matmul needs out base_partition 0; this one
                        # targets partitions 64:128 -> plain fp32 (4 cyc/row)
                        nc.tensor.matmul(
                            out=out2[64:128, :],
                            lhsT=Gsb[:, gsb_idx(row, tt_e + 1), 0:64].bitcast(F32),
                            rhs=X[:, 24 * row + tt_e + 1, 0:EMBED].bitcast(F32),
                            start=False, stop=True, skip_group_check=True)
                    osb = outsb_pool.tile([128, EMBED], F32, tag="osb")
                    p0, p1 = (32, 128) if ot == 0 else (0, 32) if ot == 12 \
                        else (0, 128)
                    # copies start at partition 0 (engine partition-base rule);
                    # ot==0 copies garbage rows 0:32 too, DMA skips them.
                    c0, c1 = (0, 32) if ot == 12 else (0, 128)
                    if ot % 2 == 0:
                        nc.vector.tensor_copy(out=osb[c0:c1, :],
                                              in_=out2[c0:c1, :])
                    else:
                        nc.scalar.copy(osb[c0:c1, :], out2[c0:c1, :])
                    base = row * (SEQ // 2) + 128 * ot - 32
                    nc.sync.dma_start(out_d.ap()[base + p0:base + p1, :],
                                      osb[p0:p1, :])

            # ---- staged pipeline ----
            for _rep in range(nrep):
                emit_gather(0)
                emit_gather(1)
                emit_scores(0)
                emit_gather(2)
                emit_scores(1)
                emit_builds(0)
                emit_big(0, list(range(0, 6)))
                emit_gather(3)
                emit_scores(2)
                emit_builds(1)
                emit_big(0, list(range(6, 13)))
                emit_scores(3)
                emit_builds(2)
                emit_big(1, list(range(0, 6)))
                emit_builds(3)
                emit_big(1, list(range(6, 13)))

    return nc


_CACHE = {}


def _get_nc(nrep=1):
    key = f"nc{nrep}"
    if key not in _CACHE:
        nc = bacc.Bacc("TRN2", target_bir_lowering=False, debug=False)
        emit_program(nc, nrep=nrep)
        nc.compile()
        _CACHE[key] = nc
    return _CACHE[key]


def prepare_in_maps(input_ids, embed_table, w_score, b_score=None):
    # b_score only shifts all 4 scores equally -> softmax-invariant; unused.
    t4pad = build_t4pad(embed_table, w_score)
    smats = build_smats()
    m2 = build_m2mask()
    idx_streams = build_idx_streams(input_ids)
    return [{"t4pad": t4pad, "idxs": idx_streams[core],
             "smats": smats, "m2m": m2} for core in range(NCORES)]


def assemble_out(results):
    outs = [results[c]["out"].reshape(BLOC, SEQ // 2, EMBED)
            for c in range(NCORES)]
    return np.concatenate(outs, axis=0)


def kernel(input_ids, embed_table, w_score, b_score):
    in_maps = prepare_in_maps(input_ids, embed_table, w_score, b_score)
    res = run_bass_kernel_spmd(_get_nc(), in_maps,
                               core_ids=list(range(NCORES)))
    return assemble_out(res.results)
